# revision 1
# baseline (speedup 1.0000x reference)
"""Trainium2 Bass kernel for nn_CppnPotentialCAStep.

Reference computation (per kernel k of NK=32):
  pot_k = depthwise_conv3d_wrap(x[:, :, :, c0[k]], kernels[k])    # 15^3 taps, wrap pad
  g_k   = exp(-(pot_k - m[k])^2 / (2 s[k]^2)) * 2 - 1
  field[c] = sum_{k: c1[k]==c} g_k
  out = clip(input + field / T, 0, 10)

Device mapping (8 NeuronCores, SPMD):
  The 3D conv becomes PE-array matmuls via a banded-Toeplitz stationary
  operand over the X axis: for an X-chunk of B outputs, W[u, (k, b)]
  holds taps w_k[u-b, dy, dz] (15-wide band in a B+14-row window) and
  the moving operand streams the wrap-padded channel grid, one column
  per (Y, Z) output point.  The 225 (dy, dz) tap planes are covered
  `ns` per matmul by stacking `ns` windows in the contraction dim, each
  window holding the channel pre-shifted in Z, so one accumulating
  matmul advances several taps at once.  Kernels sharing a source
  channel c0 are packed in the M dim.

  Two uniform subtask types (same instruction stream on every core,
  per-core data):
    A: groups of 3-4 kernels sharing c0:  B=24, ns=3 (Z-shifts 0/5/10),
       K=114, M<=96, 75 matmuls per PSUM tile, 4 X-chunks.
    B: pairs/singles:                     B=48, ns=2 (Z-shift 8),
       K=124, M<=96, 120 matmuls per PSUM tile, 2 X-chunks.
  Each (group, X-chunk) is split into two Y-halves; with the actual c0
  multiplicities this yields 56 A-halves + 24 B-halves = exactly
  7 A + 3 B per core.

  The Gaussian runs on ScalarE straight out of PSUM:
      t = Square(pot * (1/(sqrt2 s)) - m/(sqrt2 s));  g0 = Exp(-t)
  Host applies growth = 2*g0 - 1, the c1 scatter-add, /T, +input, clip.
"""

import numpy as np
import ml_dtypes

BF16 = ml_dtypes.bfloat16

S = 96          # grid size
C = 16          # channels
KS = 15         # kernel taps per axis
PAD = 7
MAXP = 10.0

M = 96          # stationary free dim (output rows) for both types
YW = 62         # slab Y extent: 48 outputs + 14 halo
YP = 110        # padded Y extent of the full grid
ZPH = 120       # host Z padding: [-7, 113) covers max shift 10 + halo
RHS_F = YW * YP  # free elements per slab partition row (Z inner = 110)
# PSUM tiles over the 48 local Y rows: 9x5 + 1x3 (480 f32 fills a bank)
YTILES = [(5 * i, 5) for i in range(9)] + [(45, 3)]
NCORES = 8

# type A: 3-4 kernels per group
BA, NSA = 24, 3
WA = BA + KS - 1            # 38
KA = NSA * WA               # 114
SH_A = (0, 5, 10)
STEPS_A = [(dy, j) for dy in range(KS) for j in range(5)]    # 75
# type B: 1-2 kernels per group
BB, NSB = 48, 2
WB = BB + KS - 1            # 62
KB = NSB * WB               # 124
SH_B = (0, 8)
STEPS_B = [(dy, j) for dy in range(KS) for j in range(8)]    # 120


def _build_groups(c0_idx):
    """Split kernels into same-channel groups: quads/triples (A), pairs/
    singles (B)."""
    by_ch = {}
    for k, c in enumerate(c0_idx):
        by_ch.setdefault(int(c), []).append(k)
    ga, gb = [], []
    for c in sorted(by_ch):
        ks = by_ch[c]
        while len(ks) >= 4:
            ga.append((c, ks[:4]))
            ks = ks[4:]
        if len(ks) == 3:
            ga.append((c, ks))
        elif ks:
            gb.append((c, ks))
    return ga, gb


def _band(v15, b):
    """[b+14, b] Toeplitz band: out[col] += v[row-col] for row-col in
    [0,15)."""
    z = np.zeros((b + KS - 1, b), np.float32)
    rows = np.arange(KS)[:, None] + np.arange(b)[None, :]
    z[rows, np.arange(b)[None, :]] = v15[:, None]
    return z


def _build_nc(nA, nB):
    import concourse.bass as bass  # noqa: F401
    import concourse.mybir as mybir
    from concourse import bacc
    from concourse.tile import TileContext

    nc = bacc.Bacc(None, target_bir_lowering=False)
    rhsA = nc.dram_tensor("rhsA", [max(nA, 1), KA, RHS_F],
                          mybir.dt.bfloat16, kind="ExternalInput")
    wtsA = nc.dram_tensor("wtsA", [max(nA, 1), KA, len(STEPS_A) * M],
                          mybir.dt.bfloat16, kind="ExternalInput")
    rhsB = nc.dram_tensor("rhsB", [max(nB, 1), KB, RHS_F],
                          mybir.dt.bfloat16, kind="ExternalInput")
    wtsB = nc.dram_tensor("wtsB", [max(nB, 1), KB, len(STEPS_B) * M],
                          mybir.dt.bfloat16, kind="ExternalInput")
    par_in = nc.dram_tensor("par", [M, 2 * (nA + nB)],
                            mybir.dt.float32, kind="ExternalInput")
    g0_out = nc.dram_tensor("g0", [nA + nB, M, 48 * S],
                            mybir.dt.float32, kind="ExternalOutput")
    AF = mybir.ActivationFunctionType
    NSUB = nA + nB

    with TileContext(nc) as tc:
        with tc.tile_pool(name="rhsp", bufs=2) as rhsp, \
             tc.tile_pool(name="wp", bufs=2) as wp, \
             tc.tile_pool(name="parp", bufs=1) as parp, \
             tc.tile_pool(name="psp", bufs=4, space="PSUM") as psp, \
             tc.tile_pool(name="gp", bufs=4) as gp:
            par_t = parp.tile([M, 2 * NSUB], mybir.dt.float32)
            nc.sync.dma_start(out=par_t, in_=par_in[:])

            def half_subtask(s, rhs_ext, wts_ext, kdim, steps):
                rhs_t = rhsp.tile([kdim, RHS_F], mybir.dt.bfloat16,
                                  tag="rhs")
                # chunked loads: chain 0 reads Y-rows 0-18 and the first
                # 1/5 of the weight columns, so the PE can start before
                # the full 3.2 MB slab lands (saves ~18 us of lead-in)
                for a, b in ((0, 24), (24, 44), (44, YW)):
                    nc.sync.dma_start(out=rhs_t[:, a * YP:b * YP],
                                      in_=rhs_ext[:, a * YP:b * YP])
                w_t = wp.tile([kdim, len(steps) * M], mybir.dt.bfloat16,
                              tag="wts")
                wq = (len(steps) // 5) * M
                for q in range(5):
                    nc.sync.dma_start(out=w_t[:, q * wq:(q + 1) * wq],
                                      in_=wts_ext[:, q * wq:(q + 1) * wq])
                rhs3 = rhs_t.rearrange("p (y z) -> p y z", z=YP)
                for y0, ny in YTILES:
                    nt = ny * S
                    ps_t = psp.tile([M, nt], mybir.dt.float32, tag="ps")
                    last = len(steps) - 1
                    for i, (dy, j) in enumerate(steps):
                        nc.tensor.matmul(
                            ps_t,
                            lhsT=w_t[:, i * M:(i + 1) * M],
                            rhs=rhs3[:, y0 + dy:y0 + dy + ny, j:j + S],
                            start=(i == 0),
                            stop=(i == last),
                        )
                    sq_t = gp.tile([M, nt], mybir.dt.float32, tag="sq")
                    nc.scalar.activation(
                        sq_t, ps_t, AF.Square,
                        bias=par_t[:, NSUB + s:NSUB + s + 1],
                        scale=par_t[:, s:s + 1])
                    g0_t = gp.tile([M, nt], mybir.dt.float32, tag="g0")
                    nc.scalar.activation(g0_t, sq_t, AF.Exp, scale=-1.0)
                    nc.sync.dma_start(
                        out=g0_out[s, :, y0 * S:(y0 + ny) * S], in_=g0_t)

            for s in range(nA):
                half_subtask(s, rhsA[s], wtsA[s], KA, STEPS_A)
            for s in range(nB):
                half_subtask(nA + s, rhsB[s], wtsB[s], KB, STEPS_B)
    nc.finalize()
    return nc


def _group_weights(kernels, ks, steps, shifts, b, kdim):
    """Banded stationary weights [kdim, len(steps)*M] for one group."""
    w = b + KS - 1
    W = np.zeros((kdim, len(steps), M), np.float32)
    for i, (dy, j) in enumerate(steps):
        for ki, k in enumerate(ks):
            cols = slice(ki * b, (ki + 1) * b)
            for blk, sh in enumerate(shifts):
                if j + sh < KS:
                    W[blk * w:(blk + 1) * w, i, cols] = _band(
                        kernels[k][:, dy, j + sh], b)
    return W.reshape(kdim, len(steps) * M).astype(BF16)


_NC_CACHE = {}
LAST_EXEC_NS = None


def kernel(input, kernels, m, s, T, c0_idx, c1_idx):
    from concourse.bass_utils import run_bass_kernel_spmd

    input = np.asarray(input, np.float32)
    kernels = np.asarray(kernels, np.float32)
    m = np.asarray(m, np.float32)
    s = np.asarray(s, np.float32)
    T = np.asarray(T, np.float32)
    c0_idx = np.asarray(c0_idx)
    c1_idx = np.asarray(c1_idx)

    x = input[0].transpose(3, 0, 1, 2)          # [C, X, Y, Z]
    ga, gb = _build_groups(c0_idx)
    if len(gb) % 2:
        gb.append(None)                          # dummy group -> even B count
    # halves: A-group -> 8 (4 X-chunks x 2 Y-halves), B-group -> 4
    halvesA = [(gi, ch, yh) for gi in range(len(ga))
               for ch in range(4) for yh in range(2)]
    halvesB = [(gi, ch, yh) for gi in range(len(gb))
               for ch in range(2) for yh in range(2)]
    nA = len(halvesA) // NCORES
    nB = len(halvesB) // NCORES
    assert nA * NCORES == len(halvesA) and nB * NCORES == len(halvesB)
    NSUB = nA + nB

    # Wrap-padded channels: [110 (X), 110 (Y), 120 (Z)]
    ip = (np.arange(YP) - PAD) % S
    iz = (np.arange(ZPH) - PAD) % S
    used = {g[0] for g in ga} | {g[0] for g in gb if g}
    padded = {c: x[c][ip][:, ip][:, :, iz].astype(BF16) for c in used}

    wA = {gi: _group_weights(kernels, ks, STEPS_A, SH_A, BA, KA)
          for gi, (c, ks) in enumerate(ga)}
    wB = {gi: _group_weights(kernels, g[1], STEPS_B, SH_B, BB, KB)
          for gi, g in enumerate(gb) if g}

    def slab(c, bx, x0, yh, shifts, w):
        """[ns*w, 62*110] moving slab: stacked Z-shifted windows."""
        P = padded[c]
        ys = 48 * yh
        out = np.empty((len(shifts) * w, RHS_F), BF16)
        for blk, sh in enumerate(shifts):
            out[blk * w:(blk + 1) * w] = \
                P[x0:x0 + w, ys:ys + YW, sh:sh + YP].reshape(w, RHS_F)
        return out

    rt2 = np.sqrt(2.0, dtype=np.float32)
    in_maps = []
    metas = []
    for core in range(NCORES):
        rhsA_h = np.zeros((max(nA, 1), KA, RHS_F), BF16)
        wtsA_h = np.zeros((max(nA, 1), KA, len(STEPS_A) * M), BF16)
        rhsB_h = np.zeros((max(nB, 1), KB, RHS_F), BF16)
        wtsB_h = np.zeros((max(nB, 1), KB, len(STEPS_B) * M), BF16)
        par_h = np.zeros((M, 2 * NSUB), np.float32)
        meta = []

        def fill(slot, gi, ch, yh, grp, bx, shifts, w, rhs_h, wts_h, wts):
            c, ks = grp
            rhs_h[:] = slab(c, bx, ch * bx, yh, shifts, w)
            wts_h[:] = wts
            for ki, k in enumerate(ks):
                sc = np.float32(1.0 / (rt2 * s[k]))
                par_h[ki * bx:(ki + 1) * bx, slot] = sc
                par_h[ki * bx:(ki + 1) * bx, NSUB + slot] = -m[k] * sc

        for j in range(nA):
            gi, ch, yh = halvesA[core * nA + j]
            fill(j, gi, ch, yh, ga[gi], BA, SH_A, WA,
                 rhsA_h[j], wtsA_h[j], wA[gi])
            meta.append(("A", ga[gi], ch, yh))
        for j in range(nB):
            gi, ch, yh = halvesB[core * nB + j]
            if gb[gi] is not None:
                fill(nA + j, gi, ch, yh, gb[gi], BB, SH_B, WB,
                     rhsB_h[j], wtsB_h[j], wB[gi])
                meta.append(("B", gb[gi], ch, yh))
            else:
                meta.append(None)
        in_maps.append({"rhsA": rhsA_h, "wtsA": wtsA_h,
                        "rhsB": rhsB_h, "wtsB": wtsB_h, "par": par_h})
        metas.append(meta)

    key = (nA, nB)
    if key not in _NC_CACHE:
        _NC_CACHE[key] = _build_nc(nA, nB)
    nc = _NC_CACHE[key]

    import os
    prof_dir = os.environ.get("KERNEL_PROFILE_DIR")
    if prof_dir:
        from trn_agent_boot.trn_boot import _ntff_profile_via_ctypes
        hook = _ntff_profile_via_ctypes("/opt/axon/libaxon_pjrt.so")
        with hook(prof_dir, [0]):
            res = run_bass_kernel_spmd(nc, in_maps,
                                       core_ids=list(range(NCORES)))
    else:
        res = run_bass_kernel_spmd(nc, in_maps, core_ids=list(range(NCORES)))
    global LAST_EXEC_NS
    LAST_EXEC_NS = res.exec_time_ns

    field = np.zeros((C, S, S, S), np.float32)      # [c, X, Y, Z]
    for core in range(NCORES):
        g0 = res.results[core]["g0"]                # [NSUB, 96, 4608]
        for j, mt in enumerate(metas[core]):
            if mt is None:
                continue
            typ, (c, ks), ch, yh = mt
            bx = BA if typ == "A" else BB
            for ki, k in enumerate(ks):
                blk = g0[j, ki * bx:(ki + 1) * bx].reshape(bx, 48, S)
                field[c1_idx[k], ch * bx:(ch + 1) * bx,
                      yh * 48:(yh + 1) * 48] += 2.0 * blk - 1.0

    out = input + field.transpose(1, 2, 3, 0)[None] / T[0]
    return np.clip(out, 0.0, MAXP).astype(np.float32)



# revision 4
# speedup vs baseline: 2.2047x; 2.2047x over previous
"""Trainium2 Bass kernel for nn_CppnPotentialCAStep — col-tiled v2.

Reference computation (per kernel k of NK=32):
  pot_k = depthwise_conv3d_wrap(x[:, :, :, c0[k]], kernels[k])    # 15^3 taps
  g_k   = exp(-(pot_k - m[k])^2 / (2 s[k]^2)) * 2 - 1
  field[c] = sum_{k: c1[k]==c} g_k
  out = clip(input + field / T, 0, 10)

Mapping: banded-Toeplitz over X (band in a b+14-row window) with ns
z-shifted copies of the channel stacked in the contraction dim, as in
v1 — but every matmul now runs in PE array mode (128, 32): 4 column
tiles execute concurrently, each streaming its own moving operand.
The 4 tiles process the 4 y-quarters (24 rows each) of the same
(channel-group, x-chunk) unit, sharing one SBUF slab and accumulating
into the 4 partition-quadrants of a shared PSUM bank, so one Square+
Exp activation pass and one output DMA cover all 4 tiles.

Unit shapes (c0 multiplicity histogram {1:6, 2:3, 3:3, 4:1, 7:1};
the 7-kernel channel is split 4+3):
  A: quads b=8 / triples b=10(+tail b=6): K=5x24=120, M=32, 45 steps
     (shifts 0,3,6,9,12; j=0..2), 64 units -> 8 per core
  P: pairs b=16: K=4x30=120, M=32, 60 steps (shifts 0,4,8,12; j=0..3),
     18 real + 6 dummy units -> 3 per core
  S: singles b=24: K=3x38=114, M=24, 75 steps (shifts 0,5,10; j=0..4),
     24 units -> 3 per core
Per core: 765 accumulation steps x 5 psum rounds x 4 concurrent tiles.
"""

import numpy as np
import ml_dtypes

BF16 = ml_dtypes.bfloat16

S = 96          # grid size
C = 16          # channels
KS = 15         # kernel taps per axis
PAD = 7
MAXP = 10.0

XP = 114        # padded x extent: orig -7 .. 106  (covers window 24 at x0=90)
YP = 110        # padded y extent: orig -7 .. 102
ZPH = 122       # padded z extent: orig -7 .. 114  (covers shift 12 + 110)
ZS = 110        # slab z extent per block
RHS_F = YP * ZS  # 12100 free elements per slab partition row
NCORES = 8
NTILES = 4      # column tiles (PE mode (128, 32))
YQ = 24         # y rows per quarter/tile
ROUNDS = [(0, 5), (5, 5), (10, 5), (15, 5), (20, 4)]  # (y0, ny) per quarter

# shape tables: (window_rows, n_blocks, shifts, t, M)
SH_A = dict(W=24, shifts=(0, 3, 6, 9, 12), t=3, M=32, K=120)
SH_P = dict(W=30, shifts=(0, 4, 8, 12), t=4, M=32, K=120)
SH_S = dict(W=38, shifts=(0, 5, 10), t=5, M=24, K=114)
for _sh in (SH_A, SH_P, SH_S):
    _sh["steps"] = [(dy, j) for dy in range(KS) for j in range(_sh["t"])]


def _build_groups(c0_idx):
    """Group kernels by source channel: quads+triples (A), pairs (P),
    singles (S). Multiplicity >=4 is peeled into quads."""
    by_ch = {}
    for k, c in enumerate(c0_idx):
        by_ch.setdefault(int(c), []).append(k)
    ga, gp, gs = [], [], []
    for c in sorted(by_ch):
        ks = by_ch[c]
        while len(ks) >= 4:
            ga.append((c, ks[:4]))
            ks = ks[4:]
        if len(ks) == 3:
            ga.append((c, ks))
        elif len(ks) == 2:
            gp.append((c, ks))
        elif len(ks) == 1:
            gs.append((c, ks))
    return ga, gp, gs


def _chunks_for(shape_key, g):
    """X-chunk widths for a group of g kernels under a shape."""
    if shape_key == "A":
        b = 8 if g == 4 else 10
        out = []
        x = 0
        while x < S:
            w = min(b, S - x)
            out.append((x, w))
            x += w
        return out                       # g=4: 12x8; g=3: 9x10 + 1x6
    if shape_key == "P":
        return [(16 * i, 16) for i in range(6)]
    return [(24 * i, 24) for i in range(4)]


def _unit_weights(kernels, ks, x_b, sh):
    """Stationary weights [K, steps*M] for one (group, chunk) unit."""
    W, shifts, t, M, K = sh["W"], sh["shifts"], sh["t"], sh["M"], sh["K"]
    steps = sh["steps"]
    b = x_b
    out = np.zeros((K, len(steps), M), np.float32)
    for i, (dy, j) in enumerate(steps):
        for blk, s in enumerate(shifts):
            dz = s + j
            if dz >= KS:
                continue
            for ki, k in enumerate(ks):
                v = kernels[k][:, dy, dz]           # [15] over dx
                for col in range(b):
                    out[blk * W + col:blk * W + col + KS, i, ki * b + col] = v
    return out.reshape(K, len(steps) * M).astype(BF16)


def _unit_slab(padded_c, x0, sh):
    """Moving slab [K, 110*110]: n_blocks z-shifted x-window copies."""
    W, shifts, K = sh["W"], sh["shifts"], sh["K"]
    out = np.empty((K, RHS_F), BF16)
    for blk, s in enumerate(shifts):
        out[blk * W:(blk + 1) * W] = \
            padded_c[x0:x0 + W, :, s:s + ZS].reshape(W, RHS_F)
    return out


def _build_nc(nA, nP, nS):
    import concourse.bass as bass  # noqa: F401
    import concourse.mybir as mybir
    from concourse import bacc
    from concourse.tile import TileContext

    nc = bacc.Bacc(None, target_bir_lowering=False)
    NU = nA + nP + nS
    rhsA = nc.dram_tensor("rhsA", [nA, SH_A["K"], RHS_F],
                          mybir.dt.bfloat16, kind="ExternalInput")
    wtsA = nc.dram_tensor("wtsA", [nA, SH_A["K"], len(SH_A["steps"]) * 32],
                          mybir.dt.bfloat16, kind="ExternalInput")
    rhsP = nc.dram_tensor("rhsP", [nP, SH_P["K"], RHS_F],
                          mybir.dt.bfloat16, kind="ExternalInput")
    wtsP = nc.dram_tensor("wtsP", [nP, SH_P["K"], len(SH_P["steps"]) * 32],
                          mybir.dt.bfloat16, kind="ExternalInput")
    rhsS = nc.dram_tensor("rhsS", [nS, SH_S["K"], RHS_F],
                          mybir.dt.bfloat16, kind="ExternalInput")
    wtsS = nc.dram_tensor("wtsS", [nS, SH_S["K"], len(SH_S["steps"]) * 24],
                          mybir.dt.bfloat16, kind="ExternalInput")
    par_in = nc.dram_tensor("par", [128, 2 * NU],
                            mybir.dt.float32, kind="ExternalInput")
    g0_out = nc.dram_tensor("g0", [NU, 128, YQ * S],
                            mybir.dt.float32, kind="ExternalOutput")
    AF = mybir.ActivationFunctionType

    with TileContext(nc) as tc:
        with tc.tile_pool(name="rhsp", bufs=2) as rhsp, \
             tc.tile_pool(name="wp", bufs=2) as wp, \
             tc.tile_pool(name="parp", bufs=1) as parp, \
             tc.tile_pool(name="psp", bufs=4, space="PSUM") as psp, \
             tc.tile_pool(name="gp", bufs=4) as gp:
            par_t = parp.tile([128, 2 * NU], mybir.dt.float32)
            nc.sync.dma_start(out=par_t, in_=par_in[:])

            def unit(u, rhs_ext, wts_ext, sh):
                K, M = sh["K"], sh["M"]
                steps = sh["steps"]
                rhs_t = rhsp.tile([K, RHS_F], mybir.dt.bfloat16, tag="rhs")
                # chunked loads so the PE can start before the full
                # slab lands
                for a, b in ((0, 36), (36, 72), (72, YP)):
                    nc.sync.dma_start(out=rhs_t[:, a * ZS:b * ZS],
                                      in_=rhs_ext[:, a * ZS:b * ZS])
                w_t = wp.tile([K, len(steps) * M], mybir.dt.bfloat16,
                              tag="wts")
                wq = (len(steps) // 5) * M
                for q in range(5):
                    nc.sync.dma_start(out=w_t[:, q * wq:(q + 1) * wq],
                                      in_=wts_ext[:, q * wq:(q + 1) * wq])
                rhs3 = rhs_t.rearrange("p (y z) -> p y z", z=ZS)
                last = len(steps) - 1
                for y0, ny in ROUNDS:
                    nt = ny * S
                    ps_t = psp.tile([128, nt], mybir.dt.float32, tag="ps")
                    for i, (dy, j) in enumerate(steps):
                        lhsT = w_t[:, i * M:(i + 1) * M]
                        for q in range(NTILES):
                            yb = q * YQ + y0 + dy
                            nc.tensor.matmul(
                                ps_t[32 * q:32 * q + M, :],
                                lhsT=lhsT,
                                rhs=rhs3[:, yb:yb + ny, j:j + S],
                                start=(i == 0),
                                stop=(i == last),
                                tile_position=(0, 32 * q),
                            )
                    sq_t = gp.tile([128, nt], mybir.dt.float32, tag="sq")
                    nc.scalar.activation(
                        sq_t, ps_t, AF.Square,
                        bias=par_t[:, NU + u:NU + u + 1],
                        scale=par_t[:, u:u + 1])
                    g0_t = gp.tile([128, nt], mybir.dt.float32, tag="g0")
                    nc.scalar.activation(g0_t, sq_t, AF.Exp, scale=-1.0)
                    nc.sync.dma_start(
                        out=g0_out[u, :, y0 * S:(y0 + ny) * S], in_=g0_t)

            for s in range(nA):
                unit(s, rhsA[s], wtsA[s], SH_A)
            for s in range(nP):
                unit(nA + s, rhsP[s], wtsP[s], SH_P)
            for s in range(nS):
                unit(nA + nP + s, rhsS[s], wtsS[s], SH_S)
    nc.finalize()
    return nc


_NC_CACHE = {}
LAST_EXEC_NS = None


def kernel(input, kernels, m, s, T, c0_idx, c1_idx):
    from concourse.bass_utils import run_bass_kernel_spmd

    input = np.asarray(input, np.float32)
    kernels = np.asarray(kernels, np.float32)
    m = np.asarray(m, np.float32)
    s = np.asarray(s, np.float32)
    T = np.asarray(T, np.float32)
    c0_idx = np.asarray(c0_idx)
    c1_idx = np.asarray(c1_idx)

    x = input[0].transpose(3, 0, 1, 2)          # [C, X, Y, Z]
    ga, gp_, gs = _build_groups(c0_idx)

    # units: (shape_key, group, x0, b) in a fixed global order
    unitsA = [("A", g, x0, b) for g in ga for x0, b in _chunks_for("A", len(g[1]))]
    unitsP = [("P", g, x0, b) for g in gp_ for x0, b in _chunks_for("P", 2)]
    unitsS = [("S", g, x0, b) for g in gs for x0, b in _chunks_for("S", 1)]
    for lst in (unitsA, unitsP, unitsS):
        while len(lst) % NCORES:
            lst.append(None)                     # dummy unit (zeros)
    nA = len(unitsA) // NCORES
    nP = len(unitsP) // NCORES
    nS = len(unitsS) // NCORES
    NU = nA + nP + nS

    # wrap-padded channels [114, 110, 122]
    ix = (np.arange(XP) - PAD) % S
    iy = (np.arange(YP) - PAD) % S
    iz = (np.arange(ZPH) - PAD) % S
    used = {g[0] for g in ga + gp_ + gs}
    padded = {c: x[c][ix][:, iy][:, :, iz].astype(BF16) for c in used}

    wcache = {}

    def unit_w(sh_key, grp, x0, b):
        key = (sh_key, grp[0], tuple(grp[1]), b)
        if key not in wcache:
            sh = {"A": SH_A, "P": SH_P, "S": SH_S}[sh_key]
            wcache[key] = _unit_weights(kernels, grp[1], b, sh)
        return wcache[key]

    rt2 = np.sqrt(2.0, dtype=np.float32)
    in_maps = []
    metas = []
    for core in range(NCORES):
        rhsA_h = np.zeros((nA, SH_A["K"], RHS_F), BF16)
        wtsA_h = np.zeros((nA, SH_A["K"], len(SH_A["steps"]) * 32), BF16)
        rhsP_h = np.zeros((max(nP, 1), SH_P["K"], RHS_F), BF16)
        wtsP_h = np.zeros((max(nP, 1), SH_P["K"], len(SH_P["steps"]) * 32),
                          BF16)
        rhsS_h = np.zeros((max(nS, 1), SH_S["K"], RHS_F), BF16)
        wtsS_h = np.zeros((max(nS, 1), SH_S["K"], len(SH_S["steps"]) * 24),
                          BF16)
        par_h = np.zeros((128, 2 * NU), np.float32)
        meta = []

        def fill(slot, local, udesc, sh, rhs_h, wts_h):
            if udesc is None:
                meta.append(None)
                return
            _, (c, ks), x0, b = udesc
            rhs_h[local] = _unit_slab(padded[c], x0, sh)
            wts_h[local] = unit_w(udesc[0], udesc[1], x0, b)
            for q in range(NTILES):
                for ki, k in enumerate(ks):
                    sc = np.float32(1.0 / (rt2 * s[k]))
                    r0 = 32 * q + ki * b
                    par_h[r0:r0 + b, slot] = sc
                    par_h[r0:r0 + b, NU + slot] = -m[k] * sc
            meta.append(udesc)

        for j in range(nA):
            fill(j, j, unitsA[core * nA + j], SH_A, rhsA_h, wtsA_h)
        for j in range(nP):
            fill(nA + j, j, unitsP[core * nP + j], SH_P, rhsP_h, wtsP_h)
        for j in range(nS):
            fill(nA + nP + j, j, unitsS[core * nS + j], SH_S,
                 rhsS_h, wtsS_h)
        in_maps.append({"rhsA": rhsA_h, "wtsA": wtsA_h,
                        "rhsP": rhsP_h, "wtsP": wtsP_h,
                        "rhsS": rhsS_h, "wtsS": wtsS_h, "par": par_h})
        metas.append(meta)

    key = (nA, nP, nS)
    if key not in _NC_CACHE:
        _NC_CACHE[key] = _build_nc(nA, max(nP, 1), max(nS, 1))
    nc = _NC_CACHE[key]

    import os
    prof_dir = os.environ.get("KERNEL_PROFILE_DIR")
    if prof_dir:
        from trn_agent_boot.trn_boot import _ntff_profile_via_ctypes
        hook = _ntff_profile_via_ctypes("/opt/axon/libaxon_pjrt.so")
        with hook(prof_dir, [0]):
            res = run_bass_kernel_spmd(nc, in_maps,
                                       core_ids=list(range(NCORES)))
    else:
        res = run_bass_kernel_spmd(nc, in_maps, core_ids=list(range(NCORES)))
    global LAST_EXEC_NS
    LAST_EXEC_NS = res.exec_time_ns

    field = np.zeros((C, S, S, S), np.float32)      # [c, X, Y, Z]
    for core in range(NCORES):
        g0 = res.results[core]["g0"]                # [NU, 128, 2304]
        for j, mt in enumerate(metas[core]):
            if mt is None:
                continue
            _, (c, ks), x0, b = mt
            blk = g0[j].reshape(128, YQ, S)
            for q in range(NTILES):
                for ki, k in enumerate(ks):
                    r0 = 32 * q + ki * b
                    field[c1_idx[k], x0:x0 + b, q * YQ:(q + 1) * YQ] += \
                        2.0 * blk[r0:r0 + b] - 1.0

    out = input + field.transpose(1, 2, 3, 0)[None] / T[0]
    return np.clip(out, 0.0, MAXP).astype(np.float32)


# revision 6
# speedup vs baseline: 2.6137x; 1.1855x over previous
"""Trainium2 Bass kernel for nn_CppnPotentialCAStep — col-tiled v2.

Reference computation (per kernel k of NK=32):
  pot_k = depthwise_conv3d_wrap(x[:, :, :, c0[k]], kernels[k])    # 15^3 taps
  g_k   = exp(-(pot_k - m[k])^2 / (2 s[k]^2)) * 2 - 1
  field[c] = sum_{k: c1[k]==c} g_k
  out = clip(input + field / T, 0, 10)

Mapping: banded-Toeplitz over X (band in a b+14-row window) with ns
z-shifted copies of the channel stacked in the contraction dim, as in
v1 — but every matmul now runs in PE array mode (128, 32): 4 column
tiles execute concurrently, each streaming its own moving operand.
The 4 tiles process the 4 y-quarters (24 rows each) of the same
(channel-group, x-chunk) unit, sharing one SBUF slab and accumulating
into the 4 partition-quadrants of a shared PSUM bank, so one Square+
Exp activation pass and one output DMA cover all 4 tiles.

Unit shapes (c0 multiplicity histogram {1:6, 2:3, 3:3, 4:1, 7:1};
the 7-kernel channel is split 4+3):
  A: quads b=8 / triples b=10(+tail b=6): K=5x24=120, M=32, 45 steps
     (shifts 0,3,6,9,12; j=0..2), 64 units -> 8 per core
  P: pairs b=16: K=4x30=120, M=32, 60 steps (shifts 0,4,8,12; j=0..3),
     18 real + 6 dummy units -> 3 per core
  S: singles b=24: K=3x38=114, M=24, 75 steps (shifts 0,5,10; j=0..4),
     24 units -> 3 per core
Per core: 765 accumulation steps x 5 psum rounds x 4 concurrent tiles.
"""

import numpy as np
import ml_dtypes

BF16 = ml_dtypes.bfloat16

S = 96          # grid size
C = 16          # channels
KS = 15         # kernel taps per axis
PAD = 7
MAXP = 10.0

XP = 114        # padded x extent: orig -7 .. 106  (covers window 24 at x0=90)
YP = 110        # padded y extent: orig -7 .. 102
ZPH = 122       # padded z extent: orig -7 .. 114  (covers shift 12 + 110)
ZS = 110        # slab z extent per block
RHS_F = YP * ZS  # 12100 free elements per slab partition row
NCORES = 8
NTILES = 4      # column tiles (PE mode (128, 32))
YQ = 24         # y rows per quarter/tile
ROUNDS = [(0, 5), (5, 5), (10, 5), (15, 5), (20, 4)]  # (y0, ny) per quarter

# shape tables: (window_rows, n_blocks, shifts, t, M)
SH_A = dict(W=24, shifts=(0, 3, 6, 9, 12), t=3, M=32, K=120)
SH_P = dict(W=30, shifts=(0, 4, 8, 12), t=4, M=32, K=120)
SH_S = dict(W=38, shifts=(0, 5, 10), t=5, M=24, K=114)
for _sh in (SH_A, SH_P, SH_S):
    _sh["steps"] = [(dy, j) for dy in range(KS) for j in range(_sh["t"])]


def _build_groups(c0_idx):
    """Group kernels by source channel: quads+triples (A), pairs (P),
    singles (S). Multiplicity >=4 is peeled into quads."""
    by_ch = {}
    for k, c in enumerate(c0_idx):
        by_ch.setdefault(int(c), []).append(k)
    ga, gp, gs = [], [], []
    for c in sorted(by_ch):
        ks = by_ch[c]
        while len(ks) >= 4:
            ga.append((c, ks[:4]))
            ks = ks[4:]
        if len(ks) == 3:
            ga.append((c, ks))
        elif len(ks) == 2:
            gp.append((c, ks))
        elif len(ks) == 1:
            gs.append((c, ks))
    return ga, gp, gs


def _chunks_for(shape_key, g):
    """X-chunk widths for a group of g kernels under a shape."""
    if shape_key == "A":
        b = 8 if g == 4 else 10
        out = []
        x = 0
        while x < S:
            w = min(b, S - x)
            out.append((x, w))
            x += w
        return out                       # g=4: 12x8; g=3: 9x10 + 1x6
    if shape_key == "P":
        return [(16 * i, 16) for i in range(6)]
    return [(24 * i, 24) for i in range(4)]


def _unit_weights(kernels, ks, x_b, sh):
    """Stationary weights [K, steps*M] for one (group, chunk) unit."""
    W, shifts, t, M, K = sh["W"], sh["shifts"], sh["t"], sh["M"], sh["K"]
    steps = sh["steps"]
    b = x_b
    out = np.zeros((K, len(steps), M), np.float32)
    for i, (dy, j) in enumerate(steps):
        for blk, s in enumerate(shifts):
            dz = s + j
            if dz >= KS:
                continue
            for ki, k in enumerate(ks):
                v = kernels[k][:, dy, dz]           # [15] over dx
                for col in range(b):
                    out[blk * W + col:blk * W + col + KS, i, ki * b + col] = v
    return out.reshape(K, len(steps) * M).astype(BF16)


def _unit_slab(padded_c, x0, sh):
    """Moving slab [K, 110*110]: n_blocks z-shifted x-window copies."""
    W, shifts, K = sh["W"], sh["shifts"], sh["K"]
    out = np.empty((K, RHS_F), BF16)
    for blk, s in enumerate(shifts):
        out[blk * W:(blk + 1) * W] = \
            padded_c[x0:x0 + W, :, s:s + ZS].reshape(W, RHS_F)
    return out


def _build_nc(nA, nP, nS):
    import concourse.bass as bass  # noqa: F401
    import concourse.mybir as mybir
    from concourse import bacc
    from concourse.tile import TileContext

    nc = bacc.Bacc(None, target_bir_lowering=False)
    NU = nA + nP + nS
    rhsA = nc.dram_tensor("rhsA", [nA, SH_A["K"], RHS_F],
                          mybir.dt.bfloat16, kind="ExternalInput")
    wtsA = nc.dram_tensor("wtsA", [nA, SH_A["K"], len(SH_A["steps"]) * 32],
                          mybir.dt.bfloat16, kind="ExternalInput")
    rhsP = nc.dram_tensor("rhsP", [nP, SH_P["K"], RHS_F],
                          mybir.dt.bfloat16, kind="ExternalInput")
    wtsP = nc.dram_tensor("wtsP", [nP, SH_P["K"], len(SH_P["steps"]) * 32],
                          mybir.dt.bfloat16, kind="ExternalInput")
    rhsS = nc.dram_tensor("rhsS", [nS, SH_S["K"], RHS_F],
                          mybir.dt.bfloat16, kind="ExternalInput")
    wtsS = nc.dram_tensor("wtsS", [nS, SH_S["K"], len(SH_S["steps"]) * 24],
                          mybir.dt.bfloat16, kind="ExternalInput")
    par_in = nc.dram_tensor("par", [128, 2 * NU],
                            mybir.dt.float32, kind="ExternalInput")
    g0_out = nc.dram_tensor("g0", [NU, 128, YQ * S],
                            mybir.dt.float32, kind="ExternalOutput")
    AF = mybir.ActivationFunctionType

    with TileContext(nc) as tc:
        with tc.tile_pool(name="rhsp", bufs=2) as rhsp, \
             tc.tile_pool(name="wp", bufs=2) as wp, \
             tc.tile_pool(name="parp", bufs=1) as parp, \
             tc.tile_pool(name="psp", bufs=4, space="PSUM") as psp, \
             tc.tile_pool(name="gp", bufs=4) as gp:
            par_t = parp.tile([128, 2 * NU], mybir.dt.float32)
            nc.sync.dma_start(out=par_t, in_=par_in[:])

            def unit(u, rhs_ext, wts_ext, sh):
                K, M = sh["K"], sh["M"]
                steps = sh["steps"]
                rhs_t = rhsp.tile([K, RHS_F], mybir.dt.bfloat16, tag="rhs")
                # chunked loads so the PE can start before the full
                # slab lands
                for a, b in ((0, 36), (36, 72), (72, YP)):
                    nc.sync.dma_start(out=rhs_t[:, a * ZS:b * ZS],
                                      in_=rhs_ext[:, a * ZS:b * ZS])
                w_t = wp.tile([K, len(steps) * M], mybir.dt.bfloat16,
                              tag="wts")
                wq = (len(steps) // 5) * M
                for q in range(5):
                    nc.sync.dma_start(out=w_t[:, q * wq:(q + 1) * wq],
                                      in_=wts_ext[:, q * wq:(q + 1) * wq])
                rhs3 = rhs_t.rearrange("p (y z) -> p y z", z=ZS)
                last = len(steps) - 1
                for y0, ny in ROUNDS:
                    nt = ny * S
                    ps_t = psp.tile([128, nt], mybir.dt.float32, tag="ps")
                    for i, (dy, j) in enumerate(steps):
                        lhsT = w_t[:, i * M:(i + 1) * M]
                        for q in range(NTILES):
                            yb = q * YQ + y0 + dy
                            nc.tensor.matmul(
                                ps_t[32 * q:32 * q + M, :],
                                lhsT=lhsT,
                                rhs=rhs3[:, yb:yb + ny, j:j + S],
                                start=(i == 0),
                                stop=(i == last),
                                tile_position=(0, 32 * q),
                            )
                    sq_t = gp.tile([128, nt], mybir.dt.float32, tag="sq")
                    nc.scalar.activation(
                        sq_t, ps_t, AF.Square,
                        bias=par_t[:, NU + u:NU + u + 1],
                        scale=par_t[:, u:u + 1])
                    g0_t = gp.tile([128, nt], mybir.dt.float32, tag="g0")
                    nc.scalar.activation(g0_t, sq_t, AF.Exp, scale=-1.0)
                    nc.sync.dma_start(
                        out=g0_out[u, :, y0 * S:(y0 + ny) * S], in_=g0_t)

            for s in range(nA):
                unit(s, rhsA[s], wtsA[s], SH_A)
            for s in range(nP):
                unit(nA + s, rhsP[s], wtsP[s], SH_P)
            for s in range(nS):
                unit(nA + nP + s, rhsS[s], wtsS[s], SH_S)
    nc.finalize()
    return nc


_NC_CACHE = {}
LAST_EXEC_NS = None


def kernel(input, kernels, m, s, T, c0_idx, c1_idx):
    from concourse.bass_utils import run_bass_kernel_spmd

    input = np.asarray(input, np.float32)
    kernels = np.asarray(kernels, np.float32)
    m = np.asarray(m, np.float32)
    s = np.asarray(s, np.float32)
    T = np.asarray(T, np.float32)
    c0_idx = np.asarray(c0_idx)
    c1_idx = np.asarray(c1_idx)

    x = input[0].transpose(3, 0, 1, 2)          # [C, X, Y, Z]
    ga, gp_, gs = _build_groups(c0_idx)

    # units: (shape_key, group, x0, b) in a fixed global order
    unitsA = [("A", g, x0, b) for g in ga for x0, b in _chunks_for("A", len(g[1]))]
    unitsP = [("P", g, x0, b) for g in gp_ for x0, b in _chunks_for("P", 2)]
    unitsS = [("S", g, x0, b) for g in gs for x0, b in _chunks_for("S", 1)]
    for lst in (unitsA, unitsP, unitsS):
        while len(lst) % NCORES:
            lst.append(None)                     # dummy unit (zeros)
    nA = len(unitsA) // NCORES
    nP = len(unitsP) // NCORES
    nS = len(unitsS) // NCORES
    NU = nA + nP + nS

    # wrap-padded channels [114, 110, 122]
    ix = (np.arange(XP) - PAD) % S
    iy = (np.arange(YP) - PAD) % S
    iz = (np.arange(ZPH) - PAD) % S
    used = {g[0] for g in ga + gp_ + gs}
    padded = {c: x[c][ix][:, iy][:, :, iz].astype(BF16) for c in used}

    wcache = {}

    def unit_w(sh_key, grp, x0, b):
        key = (sh_key, grp[0], tuple(grp[1]), b)
        if key not in wcache:
            sh = {"A": SH_A, "P": SH_P, "S": SH_S}[sh_key]
            wcache[key] = _unit_weights(kernels, grp[1], b, sh)
        return wcache[key]

    rt2 = np.sqrt(2.0, dtype=np.float32)
    in_maps = []
    metas = []
    for core in range(NCORES):
        rhsA_h = np.zeros((nA, SH_A["K"], RHS_F), BF16)
        wtsA_h = np.zeros((nA, SH_A["K"], len(SH_A["steps"]) * 32), BF16)
        rhsP_h = np.zeros((max(nP, 1), SH_P["K"], RHS_F), BF16)
        wtsP_h = np.zeros((max(nP, 1), SH_P["K"], len(SH_P["steps"]) * 32),
                          BF16)
        rhsS_h = np.zeros((max(nS, 1), SH_S["K"], RHS_F), BF16)
        wtsS_h = np.zeros((max(nS, 1), SH_S["K"], len(SH_S["steps"]) * 24),
                          BF16)
        par_h = np.zeros((128, 2 * NU), np.float32)
        meta = []

        def fill(slot, local, udesc, sh, rhs_h, wts_h):
            if udesc is None:
                meta.append(None)
                return
            _, (c, ks), x0, b = udesc
            rhs_h[local] = _unit_slab(padded[c], x0, sh)
            wts_h[local] = unit_w(udesc[0], udesc[1], x0, b)
            for q in range(NTILES):
                for ki, k in enumerate(ks):
                    sc = np.float32(1.0 / (rt2 * s[k]))
                    r0 = 32 * q + ki * b
                    par_h[r0:r0 + b, slot] = sc
                    par_h[r0:r0 + b, NU + slot] = -m[k] * sc
            meta.append(udesc)

        for j in range(nA):
            fill(j, j, unitsA[core * nA + j], SH_A, rhsA_h, wtsA_h)
        for j in range(nP):
            fill(nA + j, j, unitsP[core * nP + j], SH_P, rhsP_h, wtsP_h)
        for j in range(nS):
            fill(nA + nP + j, j, unitsS[core * nS + j], SH_S,
                 rhsS_h, wtsS_h)
        in_maps.append({"rhsA": rhsA_h, "wtsA": wtsA_h,
                        "rhsP": rhsP_h, "wtsP": wtsP_h,
                        "rhsS": rhsS_h, "wtsS": wtsS_h, "par": par_h})
        metas.append(meta)

    key = (nA, nP, nS)
    if key not in _NC_CACHE:
        _NC_CACHE[key] = _build_nc(nA, max(nP, 1), max(nS, 1))
    nc = _NC_CACHE[key]

    import os
    prof_dir = os.environ.get("KERNEL_PROFILE_DIR")
    if prof_dir:
        from trn_agent_boot.trn_boot import _ntff_profile_via_ctypes
        hook = _ntff_profile_via_ctypes("/opt/axon/libaxon_pjrt.so")
        with hook(prof_dir, [0]):
            res = run_bass_kernel_spmd(nc, in_maps,
                                       core_ids=list(range(NCORES)))
    else:
        res = run_bass_kernel_spmd(nc, in_maps, core_ids=list(range(NCORES)))
    global LAST_EXEC_NS
    LAST_EXEC_NS = res.exec_time_ns

    field = np.zeros((C, S, S, S), np.float32)      # [c, X, Y, Z]
    for core in range(NCORES):
        g0 = res.results[core]["g0"]                # [NU, 128, 2304]
        for j, mt in enumerate(metas[core]):
            if mt is None:
                continue
            _, (c, ks), x0, b = mt
            blk = g0[j].reshape(128, YQ, S)
            for q in range(NTILES):
                for ki, k in enumerate(ks):
                    r0 = 32 * q + ki * b
                    field[c1_idx[k], x0:x0 + b, q * YQ:(q + 1) * YQ] += \
                        2.0 * blk[r0:r0 + b] - 1.0

    out = input + field.transpose(1, 2, 3, 0)[None] / T[0]
    return np.clip(out, 0.0, MAXP).astype(np.float32)


# revision 7
# speedup vs baseline: 2.6523x; 1.0148x over previous
"""Trainium2 Bass kernel for nn_CppnPotentialCAStep — col-tiled v2.

Reference computation (per kernel k of NK=32):
  pot_k = depthwise_conv3d_wrap(x[:, :, :, c0[k]], kernels[k])    # 15^3 taps
  g_k   = exp(-(pot_k - m[k])^2 / (2 s[k]^2)) * 2 - 1
  field[c] = sum_{k: c1[k]==c} g_k
  out = clip(input + field / T, 0, 10)

Mapping: banded-Toeplitz over X (band in a b+14-row window) with ns
z-shifted copies of the channel stacked in the contraction dim, as in
v1 — but every matmul now runs in PE array mode (128, 32): 4 column
tiles execute concurrently, each streaming its own moving operand.
The 4 tiles process the 4 y-quarters (24 rows each) of the same
(channel-group, x-chunk) unit, sharing one SBUF slab and accumulating
into the 4 partition-quadrants of a shared PSUM bank, so one Square+
Exp activation pass and one output DMA cover all 4 tiles.

Unit shapes (c0 multiplicity histogram {1:6, 2:3, 3:3, 4:1, 7:1};
the 7-kernel channel is split 4+3):
  A: quads b=8 / triples b=10(+tail b=6): K=5x24=120, M=32, 45 steps
     (shifts 0,3,6,9,12; j=0..2), 64 units -> 8 per core
  P: pairs b=16: K=4x30=120, M=32, 60 steps (shifts 0,4,8,12; j=0..3),
     18 real + 6 dummy units -> 3 per core
  S: singles b=24: K=3x38=114, M=24, 75 steps (shifts 0,5,10; j=0..4),
     24 units -> 3 per core
Per core: 765 accumulation steps x 5 psum rounds x 4 concurrent tiles.
"""

import numpy as np
import ml_dtypes

BF16 = ml_dtypes.bfloat16

S = 96          # grid size
C = 16          # channels
KS = 15         # kernel taps per axis
PAD = 7
MAXP = 10.0

XP = 114        # padded x extent: orig -7 .. 106  (covers window 24 at x0=90)
YP = 110        # padded y extent: orig -7 .. 102
ZPH = 122       # padded z extent: orig -7 .. 114  (covers shift 12 + 110)
ZS = 110        # slab z extent per block
RHS_F = YP * ZS  # 12100 free elements per slab partition row
NCORES = 8
NTILES = 4      # column tiles (PE mode (128, 32))
YQ = 24         # y rows per quarter/tile
ROUNDS = [(0, 5), (5, 5), (10, 5), (15, 5), (20, 4)]  # (y0, ny) per quarter

# shape tables: (window_rows, n_blocks, shifts, t, M)
SH_A = dict(W=24, shifts=(0, 3, 6, 9, 12), t=3, M=32, K=120)
SH_P = dict(W=30, shifts=(0, 4, 8, 12), t=4, M=32, K=120)
SH_S = dict(W=38, shifts=(0, 5, 10), t=5, M=24, K=114)
for _sh in (SH_A, SH_P, SH_S):
    _sh["steps"] = [(dy, j) for dy in range(KS) for j in range(_sh["t"])]


def _build_groups(c0_idx):
    """Group kernels by source channel: quads+triples (A), pairs (P),
    singles (S). Multiplicity >=4 is peeled into quads."""
    by_ch = {}
    for k, c in enumerate(c0_idx):
        by_ch.setdefault(int(c), []).append(k)
    ga, gp, gs = [], [], []
    for c in sorted(by_ch):
        ks = by_ch[c]
        while len(ks) >= 4:
            ga.append((c, ks[:4]))
            ks = ks[4:]
        if len(ks) == 3:
            ga.append((c, ks))
        elif len(ks) == 2:
            gp.append((c, ks))
        elif len(ks) == 1:
            gs.append((c, ks))
    return ga, gp, gs


def _chunks_for(shape_key, g):
    """X-chunk widths for a group of g kernels under a shape."""
    if shape_key == "A":
        b = 8 if g == 4 else 10
        out = []
        x = 0
        while x < S:
            w = min(b, S - x)
            out.append((x, w))
            x += w
        return out                       # g=4: 12x8; g=3: 9x10 + 1x6
    if shape_key == "P":
        return [(16 * i, 16) for i in range(6)]
    return [(24 * i, 24) for i in range(4)]


def _unit_weights(kernels, ks, x_b, sh):
    """Stationary weights [K, steps*M] for one (group, chunk) unit."""
    W, shifts, t, M, K = sh["W"], sh["shifts"], sh["t"], sh["M"], sh["K"]
    steps = sh["steps"]
    b = x_b
    out = np.zeros((K, len(steps), M), np.float32)
    for i, (dy, j) in enumerate(steps):
        for blk, s in enumerate(shifts):
            dz = s + j
            if dz >= KS:
                continue
            for ki, k in enumerate(ks):
                v = kernels[k][:, dy, dz]           # [15] over dx
                for col in range(b):
                    out[blk * W + col:blk * W + col + KS, i, ki * b + col] = v
    return out.reshape(K, len(steps) * M).astype(BF16)


def _unit_slab(padded_c, x0, sh):
    """Moving slab [K, 110*110]: n_blocks z-shifted x-window copies."""
    W, shifts, K = sh["W"], sh["shifts"], sh["K"]
    out = np.empty((K, RHS_F), BF16)
    for blk, s in enumerate(shifts):
        out[blk * W:(blk + 1) * W] = \
            padded_c[x0:x0 + W, :, s:s + ZS].reshape(W, RHS_F)
    return out


def _build_nc(nA, nP, nS):
    import concourse.bass as bass  # noqa: F401
    import concourse.mybir as mybir
    from concourse import bacc
    from concourse.tile import TileContext

    nc = bacc.Bacc(None, target_bir_lowering=False)
    NU = nA + nP + nS
    rhsA = nc.dram_tensor("rhsA", [nA, SH_A["K"], RHS_F],
                          mybir.dt.bfloat16, kind="ExternalInput")
    wtsA = nc.dram_tensor("wtsA", [nA, SH_A["K"], len(SH_A["steps"]) * 32],
                          mybir.dt.bfloat16, kind="ExternalInput")
    rhsP = nc.dram_tensor("rhsP", [nP, SH_P["K"], RHS_F],
                          mybir.dt.bfloat16, kind="ExternalInput")
    wtsP = nc.dram_tensor("wtsP", [nP, SH_P["K"], len(SH_P["steps"]) * 32],
                          mybir.dt.bfloat16, kind="ExternalInput")
    rhsS = nc.dram_tensor("rhsS", [nS, SH_S["K"], RHS_F],
                          mybir.dt.bfloat16, kind="ExternalInput")
    wtsS = nc.dram_tensor("wtsS", [nS, SH_S["K"], len(SH_S["steps"]) * 24],
                          mybir.dt.bfloat16, kind="ExternalInput")
    par_in = nc.dram_tensor("par", [128, 2 * NU],
                            mybir.dt.float32, kind="ExternalInput")
    g0_out = nc.dram_tensor("g0", [NU, 128, YQ * S],
                            mybir.dt.float32, kind="ExternalOutput")
    AF = mybir.ActivationFunctionType

    with TileContext(nc) as tc:
        with tc.tile_pool(name="rhsp", bufs=2) as rhsp, \
             tc.tile_pool(name="wp", bufs=2) as wp, \
             tc.tile_pool(name="parp", bufs=1) as parp, \
             tc.tile_pool(name="psp", bufs=4, space="PSUM") as psp, \
             tc.tile_pool(name="gp", bufs=4) as gp:
            par_t = parp.tile([128, 2 * NU], mybir.dt.float32)
            nc.sync.dma_start(out=par_t, in_=par_in[:])

            def load(rhs_ext, wts_ext, sh):
                K, M = sh["K"], sh["M"]
                nst = len(sh["steps"])
                w_t = wp.tile([K, nst * M], mybir.dt.bfloat16, tag="wts")
                wq = (nst // 5) * M
                for q in range(5):
                    eng = nc.scalar if q % 2 else nc.sync
                    eng.dma_start(out=w_t[:, q * wq:(q + 1) * wq],
                                  in_=wts_ext[:, q * wq:(q + 1) * wq])
                rhs_t = rhsp.tile([K, RHS_F], mybir.dt.bfloat16, tag="rhs")
                # chunked loads on both hwdge queues so the PE can start
                # before the full slab lands
                ys = (0, 18, 37, 55, 73, 91, YP)
                for ci in range(6):
                    a, b = ys[ci], ys[ci + 1]
                    eng = nc.scalar if ci % 2 else nc.sync
                    eng.dma_start(out=rhs_t[:, a * ZS:b * ZS],
                                  in_=rhs_ext[:, a * ZS:b * ZS])
                return rhs_t, w_t

            def rounds(u, handles, sh, r_list):
                K, M = sh["K"], sh["M"]
                steps = sh["steps"]
                rhs_t, w_t = handles
                rhs3 = rhs_t.rearrange("p (y z) -> p y z", z=ZS)
                last = len(steps) - 1
                for y0, ny in r_list:
                    nt = ny * S
                    ps_t = psp.tile([128, nt], mybir.dt.float32, tag="ps")
                    for i, (dy, j) in enumerate(steps):
                        lhsT = w_t[:, i * M:(i + 1) * M]
                        for q in range(NTILES):
                            yb = q * YQ + y0 + dy
                            nc.tensor.matmul(
                                ps_t[32 * q:32 * q + M, :],
                                lhsT=lhsT,
                                rhs=rhs3[:, yb:yb + ny, j:j + S],
                                start=(i == 0),
                                stop=(i == last),
                                tile_position=(0, 32 * q),
                            )
                    sq_t = gp.tile([128, nt], mybir.dt.float32, tag="sq")
                    nc.scalar.activation(
                        sq_t, ps_t, AF.Square,
                        bias=par_t[:, NU + u:NU + u + 1],
                        scale=par_t[:, u:u + 1])
                    g0_t = gp.tile([128, nt], mybir.dt.float32, tag="g0")
                    nc.scalar.activation(g0_t, sq_t, AF.Exp, scale=-1.0)
                    nc.sync.dma_start(
                        out=g0_out[u, :, y0 * S:(y0 + ny) * S], in_=g0_t)

            seq = [(rhsA[s], wtsA[s], SH_A) for s in range(nA)] + \
                  [(rhsP[s], wtsP[s], SH_P) for s in range(nP)] + \
                  [(rhsS[s], wtsS[s], SH_S) for s in range(nS)]
            handles = load(*seq[0])
            for u in range(len(seq)):
                sh = seq[u][2]
                # first psum round, then prefetch the next unit's data
                # while the remaining rounds keep the PE busy
                rounds(u, handles, sh, ROUNDS[:1])
                nxt = load(*seq[u + 1]) if u + 1 < len(seq) else None
                rounds(u, handles, sh, ROUNDS[1:])
                handles = nxt
    nc.finalize()
    return nc


_NC_CACHE = {}
LAST_EXEC_NS = None


def kernel(input, kernels, m, s, T, c0_idx, c1_idx):
    from concourse.bass_utils import run_bass_kernel_spmd

    input = np.asarray(input, np.float32)
    kernels = np.asarray(kernels, np.float32)
    m = np.asarray(m, np.float32)
    s = np.asarray(s, np.float32)
    T = np.asarray(T, np.float32)
    c0_idx = np.asarray(c0_idx)
    c1_idx = np.asarray(c1_idx)

    x = input[0].transpose(3, 0, 1, 2)          # [C, X, Y, Z]
    ga, gp_, gs = _build_groups(c0_idx)

    # units: (shape_key, group, x0, b) in a fixed global order
    unitsA = [("A", g, x0, b) for g in ga for x0, b in _chunks_for("A", len(g[1]))]
    unitsP = [("P", g, x0, b) for g in gp_ for x0, b in _chunks_for("P", 2)]
    unitsS = [("S", g, x0, b) for g in gs for x0, b in _chunks_for("S", 1)]
    for lst in (unitsA, unitsP, unitsS):
        while len(lst) % NCORES:
            lst.append(None)                     # dummy unit (zeros)
    nA = len(unitsA) // NCORES
    nP = len(unitsP) // NCORES
    nS = len(unitsS) // NCORES
    NU = nA + nP + nS

    # wrap-padded channels [114, 110, 122]
    ix = (np.arange(XP) - PAD) % S
    iy = (np.arange(YP) - PAD) % S
    iz = (np.arange(ZPH) - PAD) % S
    used = {g[0] for g in ga + gp_ + gs}
    padded = {c: x[c][ix][:, iy][:, :, iz].astype(BF16) for c in used}

    wcache = {}

    def unit_w(sh_key, grp, x0, b):
        key = (sh_key, grp[0], tuple(grp[1]), b)
        if key not in wcache:
            sh = {"A": SH_A, "P": SH_P, "S": SH_S}[sh_key]
            wcache[key] = _unit_weights(kernels, grp[1], b, sh)
        return wcache[key]

    rt2 = np.sqrt(2.0, dtype=np.float32)
    in_maps = []
    metas = []
    for core in range(NCORES):
        rhsA_h = np.zeros((nA, SH_A["K"], RHS_F), BF16)
        wtsA_h = np.zeros((nA, SH_A["K"], len(SH_A["steps"]) * 32), BF16)
        rhsP_h = np.zeros((max(nP, 1), SH_P["K"], RHS_F), BF16)
        wtsP_h = np.zeros((max(nP, 1), SH_P["K"], len(SH_P["steps"]) * 32),
                          BF16)
        rhsS_h = np.zeros((max(nS, 1), SH_S["K"], RHS_F), BF16)
        wtsS_h = np.zeros((max(nS, 1), SH_S["K"], len(SH_S["steps"]) * 24),
                          BF16)
        par_h = np.zeros((128, 2 * NU), np.float32)
        meta = []

        def fill(slot, local, udesc, sh, rhs_h, wts_h):
            if udesc is None:
                meta.append(None)
                return
            _, (c, ks), x0, b = udesc
            rhs_h[local] = _unit_slab(padded[c], x0, sh)
            wts_h[local] = unit_w(udesc[0], udesc[1], x0, b)
            for q in range(NTILES):
                for ki, k in enumerate(ks):
                    sc = np.float32(1.0 / (rt2 * s[k]))
                    r0 = 32 * q + ki * b
                    par_h[r0:r0 + b, slot] = sc
                    par_h[r0:r0 + b, NU + slot] = -m[k] * sc
            meta.append(udesc)

        for j in range(nA):
            fill(j, j, unitsA[core * nA + j], SH_A, rhsA_h, wtsA_h)
        for j in range(nP):
            fill(nA + j, j, unitsP[core * nP + j], SH_P, rhsP_h, wtsP_h)
        for j in range(nS):
            fill(nA + nP + j, j, unitsS[core * nS + j], SH_S,
                 rhsS_h, wtsS_h)
        in_maps.append({"rhsA": rhsA_h, "wtsA": wtsA_h,
                        "rhsP": rhsP_h, "wtsP": wtsP_h,
                        "rhsS": rhsS_h, "wtsS": wtsS_h, "par": par_h})
        metas.append(meta)

    key = (nA, nP, nS)
    if key not in _NC_CACHE:
        _NC_CACHE[key] = _build_nc(nA, max(nP, 1), max(nS, 1))
    nc = _NC_CACHE[key]

    import os
    prof_dir = os.environ.get("KERNEL_PROFILE_DIR")
    if prof_dir:
        from trn_agent_boot.trn_boot import _ntff_profile_via_ctypes
        hook = _ntff_profile_via_ctypes("/opt/axon/libaxon_pjrt.so")
        with hook(prof_dir, [0]):
            res = run_bass_kernel_spmd(nc, in_maps,
                                       core_ids=list(range(NCORES)))
    else:
        res = run_bass_kernel_spmd(nc, in_maps, core_ids=list(range(NCORES)))
    global LAST_EXEC_NS
    LAST_EXEC_NS = res.exec_time_ns

    field = np.zeros((C, S, S, S), np.float32)      # [c, X, Y, Z]
    for core in range(NCORES):
        g0 = res.results[core]["g0"]                # [NU, 128, 2304]
        for j, mt in enumerate(metas[core]):
            if mt is None:
                continue
            _, (c, ks), x0, b = mt
            blk = g0[j].reshape(128, YQ, S)
            for q in range(NTILES):
                for ki, k in enumerate(ks):
                    r0 = 32 * q + ki * b
                    field[c1_idx[k], x0:x0 + b, q * YQ:(q + 1) * YQ] += \
                        2.0 * blk[r0:r0 + b] - 1.0

    out = input + field.transpose(1, 2, 3, 0)[None] / T[0]
    return np.clip(out, 0.0, MAXP).astype(np.float32)


# revision 11
# speedup vs baseline: 2.6569x; 1.0017x over previous
"""Trainium2 Bass kernel for nn_CppnPotentialCAStep — col-tiled v2.

Reference computation (per kernel k of NK=32):
  pot_k = depthwise_conv3d_wrap(x[:, :, :, c0[k]], kernels[k])    # 15^3 taps
  g_k   = exp(-(pot_k - m[k])^2 / (2 s[k]^2)) * 2 - 1
  field[c] = sum_{k: c1[k]==c} g_k
  out = clip(input + field / T, 0, 10)

Mapping: banded-Toeplitz over X (band in a b+14-row window) with ns
z-shifted copies of the channel stacked in the contraction dim, as in
v1 — but every matmul now runs in PE array mode (128, 32): 4 column
tiles execute concurrently, each streaming its own moving operand.
The 4 tiles process the 4 y-quarters (24 rows each) of the same
(channel-group, x-chunk) unit, sharing one SBUF slab and accumulating
into the 4 partition-quadrants of a shared PSUM bank, so one Square+
Exp activation pass and one output DMA cover all 4 tiles.

Unit shapes (c0 multiplicity histogram {1:6, 2:3, 3:3, 4:1, 7:1};
the 7-kernel channel is split 4+3):
  A: quads b=8 / triples b=10(+tail b=6): K=5x24=120, M=32, 45 steps
     (shifts 0,3,6,9,12; j=0..2), 64 units -> 8 per core
  P: pairs b=16: K=4x30=120, M=32, 60 steps (shifts 0,4,8,12; j=0..3),
     18 real + 6 dummy units -> 3 per core
  S: singles b=24: K=3x38=114, M=24, 75 steps (shifts 0,5,10; j=0..4),
     24 units -> 3 per core
Per core: 765 accumulation steps x 5 psum rounds x 4 concurrent tiles.
"""

import numpy as np
import ml_dtypes

BF16 = ml_dtypes.bfloat16

S = 96          # grid size
C = 16          # channels
KS = 15         # kernel taps per axis
PAD = 7
MAXP = 10.0

XP = 114        # padded x extent: orig -7 .. 106  (covers window 24 at x0=90)
YP = 110        # padded y extent: orig -7 .. 102
ZPH = 122       # padded z extent: orig -7 .. 114  (covers shift 12 + 110)
ZS = 110        # slab z extent per block
RHS_F = YP * ZS  # 12100 free elements per slab partition row
NCORES = 8
NTILES = 4      # column tiles (PE mode (128, 32))
YQ = 24         # y rows per quarter/tile
ROUNDS = [(0, 5), (5, 5), (10, 5), (15, 5), (20, 4)]  # (y0, ny) per quarter

# shape tables: (window_rows, n_blocks, shifts, t, M)
SH_A = dict(W=24, shifts=(0, 3, 6, 9, 12), t=3, M=32, K=120)
SH_P = dict(W=30, shifts=(0, 4, 8, 12), t=4, M=32, K=120)
SH_S = dict(W=38, shifts=(0, 5, 10), t=5, M=24, K=114)
for _sh in (SH_A, SH_P, SH_S):
    _sh["steps"] = [(dy, j) for dy in range(KS) for j in range(_sh["t"])]


def _build_groups(c0_idx):
    """Group kernels by source channel: quads+triples (A), pairs (P),
    singles (S). Multiplicity >=4 is peeled into quads."""
    by_ch = {}
    for k, c in enumerate(c0_idx):
        by_ch.setdefault(int(c), []).append(k)
    ga, gp, gs = [], [], []
    for c in sorted(by_ch):
        ks = by_ch[c]
        while len(ks) >= 4:
            ga.append((c, ks[:4]))
            ks = ks[4:]
        if len(ks) == 3:
            ga.append((c, ks))
        elif len(ks) == 2:
            gp.append((c, ks))
        elif len(ks) == 1:
            gs.append((c, ks))
    return ga, gp, gs


def _chunks_for(shape_key, g):
    """X-chunk widths for a group of g kernels under a shape."""
    if shape_key == "A":
        b = 8 if g == 4 else 10
        out = []
        x = 0
        while x < S:
            w = min(b, S - x)
            out.append((x, w))
            x += w
        return out                       # g=4: 12x8; g=3: 9x10 + 1x6
    if shape_key == "P":
        return [(16 * i, 16) for i in range(6)]
    return [(24 * i, 24) for i in range(4)]


def _unit_weights(kernels, ks, x_b, sh):
    """Stationary weights [K, steps*M] for one (group, chunk) unit."""
    W, shifts, t, M, K = sh["W"], sh["shifts"], sh["t"], sh["M"], sh["K"]
    steps = sh["steps"]
    b = x_b
    out = np.zeros((K, len(steps), M), np.float32)
    for i, (dy, j) in enumerate(steps):
        for blk, s in enumerate(shifts):
            dz = s + j
            if dz >= KS:
                continue
            for ki, k in enumerate(ks):
                v = kernels[k][:, dy, dz]           # [15] over dx
                for col in range(b):
                    out[blk * W + col:blk * W + col + KS, i, ki * b + col] = v
    return out.reshape(K, len(steps) * M).astype(BF16)


def _unit_slab(padded_c, x0, sh):
    """Moving slab [K, 110*110]: n_blocks z-shifted x-window copies."""
    W, shifts, K = sh["W"], sh["shifts"], sh["K"]
    out = np.empty((K, RHS_F), BF16)
    for blk, s in enumerate(shifts):
        out[blk * W:(blk + 1) * W] = \
            padded_c[x0:x0 + W, :, s:s + ZS].reshape(W, RHS_F)
    return out


def _build_nc(nA, nP, nS):
    import concourse.bass as bass  # noqa: F401
    import concourse.mybir as mybir
    from concourse import bacc
    from concourse.tile import TileContext

    nc = bacc.Bacc(None, target_bir_lowering=False)
    NU = nA + nP + nS
    rhsA = nc.dram_tensor("rhsA", [nA, SH_A["K"], RHS_F],
                          mybir.dt.bfloat16, kind="ExternalInput")
    wtsA = nc.dram_tensor("wtsA", [nA, SH_A["K"], len(SH_A["steps"]) * 32],
                          mybir.dt.bfloat16, kind="ExternalInput")
    rhsP = nc.dram_tensor("rhsP", [nP, SH_P["K"], RHS_F],
                          mybir.dt.bfloat16, kind="ExternalInput")
    wtsP = nc.dram_tensor("wtsP", [nP, SH_P["K"], len(SH_P["steps"]) * 32],
                          mybir.dt.bfloat16, kind="ExternalInput")
    rhsS = nc.dram_tensor("rhsS", [nS, SH_S["K"], RHS_F],
                          mybir.dt.bfloat16, kind="ExternalInput")
    wtsS = nc.dram_tensor("wtsS", [nS, SH_S["K"], len(SH_S["steps"]) * 24],
                          mybir.dt.bfloat16, kind="ExternalInput")
    par_in = nc.dram_tensor("par", [128, 2 * NU],
                            mybir.dt.float32, kind="ExternalInput")
    g0_out = nc.dram_tensor("g0", [NU, 128, YQ * S],
                            mybir.dt.float32, kind="ExternalOutput")
    AF = mybir.ActivationFunctionType

    with TileContext(nc) as tc:
        with tc.tile_pool(name="rhsp", bufs=2) as rhsp, \
             tc.tile_pool(name="wp", bufs=2) as wp, \
             tc.tile_pool(name="parp", bufs=1) as parp, \
             tc.tile_pool(name="psp", bufs=4, space="PSUM") as psp, \
             tc.tile_pool(name="gp", bufs=4) as gp:
            par_t = parp.tile([128, 2 * NU], mybir.dt.float32)
            nc.sync.dma_start(out=par_t, in_=par_in[:])

            def load(rhs_ext, wts_ext, sh):
                K, M = sh["K"], sh["M"]
                nst = len(sh["steps"])
                w_t = wp.tile([K, nst * M], mybir.dt.bfloat16, tag="wts")
                wq = (nst // 5) * M
                for q in range(5):
                    eng = nc.scalar if q % 2 else nc.sync
                    eng.dma_start(out=w_t[:, q * wq:(q + 1) * wq],
                                  in_=wts_ext[:, q * wq:(q + 1) * wq])
                rhs_t = rhsp.tile([K, RHS_F], mybir.dt.bfloat16, tag="rhs")
                # chunked loads on both hwdge queues; with the y-mod-4
                # quadrant interleave, psum round r only needs y rows up
                # to 20r+33, so compute starts after the first chunks
                ys = (0, 18, 34, 54, 74, 94, YP)
                for ci in range(6):
                    a, b = ys[ci], ys[ci + 1]
                    eng = nc.scalar if ci % 2 else nc.sync
                    eng.dma_start(out=rhs_t[:, a * ZS:b * ZS],
                                  in_=rhs_ext[:, a * ZS:b * ZS])
                return rhs_t, w_t

            def rounds(u, handles, sh, r_list):
                K, M = sh["K"], sh["M"]
                steps = sh["steps"]
                rhs_t, w_t = handles
                rhs3 = rhs_t.rearrange("p (y z) -> p y z", z=ZS)
                last = len(steps) - 1
                for y0, ny in r_list:
                    nt = ny * S
                    ps_t = psp.tile([128, nt], mybir.dt.float32, tag="ps")
                    for i, (dy, j) in enumerate(steps):
                        lhsT = w_t[:, i * M:(i + 1) * M]
                        for q in range(NTILES):
                            # quadrant q covers output y rows q, q+4, ...
                            yb = 4 * y0 + q + dy
                            nc.tensor.matmul(
                                ps_t[32 * q:32 * q + M, :],
                                lhsT=lhsT,
                                rhs=rhs3[:, yb:yb + 4 * ny - 3:4,
                                         j:j + S],
                                start=(i == 0),
                                stop=(i == last),
                                tile_position=(0, 32 * q),
                            )
                    sq_t = gp.tile([128, nt], mybir.dt.float32, tag="sq")
                    nc.scalar.activation(
                        sq_t, ps_t, AF.Square,
                        bias=par_t[:, NU + u:NU + u + 1],
                        scale=par_t[:, u:u + 1])
                    g0_t = gp.tile([128, nt], mybir.dt.float32, tag="g0")
                    nc.scalar.activation(g0_t, sq_t, AF.Exp, scale=-1.0)
                    nc.sync.dma_start(
                        out=g0_out[u, :, y0 * S:(y0 + ny) * S], in_=g0_t)

            seq = [(rhsA[s], wtsA[s], SH_A) for s in range(nA)] + \
                  [(rhsP[s], wtsP[s], SH_P) for s in range(nP)] + \
                  [(rhsS[s], wtsS[s], SH_S) for s in range(nS)]
            handles = load(*seq[0])
            for u in range(len(seq)):
                sh = seq[u][2]
                # first psum round, then prefetch the next unit's data
                # while the remaining rounds keep the PE busy
                rounds(u, handles, sh, ROUNDS[:1])
                nxt = load(*seq[u + 1]) if u + 1 < len(seq) else None
                rounds(u, handles, sh, ROUNDS[1:])
                handles = nxt
    nc.finalize()
    return nc


_NC_CACHE = {}
LAST_EXEC_NS = None


def kernel(input, kernels, m, s, T, c0_idx, c1_idx):
    from concourse.bass_utils import run_bass_kernel_spmd

    input = np.asarray(input, np.float32)
    kernels = np.asarray(kernels, np.float32)
    m = np.asarray(m, np.float32)
    s = np.asarray(s, np.float32)
    T = np.asarray(T, np.float32)
    c0_idx = np.asarray(c0_idx)
    c1_idx = np.asarray(c1_idx)

    x = input[0].transpose(3, 0, 1, 2)          # [C, X, Y, Z]
    ga, gp_, gs = _build_groups(c0_idx)

    # units: (shape_key, group, x0, b) in a fixed global order
    unitsA = [("A", g, x0, b) for g in ga for x0, b in _chunks_for("A", len(g[1]))]
    unitsP = [("P", g, x0, b) for g in gp_ for x0, b in _chunks_for("P", 2)]
    unitsS = [("S", g, x0, b) for g in gs for x0, b in _chunks_for("S", 1)]
    for lst in (unitsA, unitsP, unitsS):
        while len(lst) % NCORES:
            lst.append(None)                     # dummy unit (zeros)
    nA = len(unitsA) // NCORES
    nP = len(unitsP) // NCORES
    nS = len(unitsS) // NCORES
    NU = nA + nP + nS

    # wrap-padded channels [114, 110, 122]
    ix = (np.arange(XP) - PAD) % S
    iy = (np.arange(YP) - PAD) % S
    iz = (np.arange(ZPH) - PAD) % S
    used = {g[0] for g in ga + gp_ + gs}
    padded = {c: x[c][ix][:, iy][:, :, iz].astype(BF16) for c in used}

    wcache = {}

    def unit_w(sh_key, grp, x0, b):
        key = (sh_key, grp[0], tuple(grp[1]), b)
        if key not in wcache:
            sh = {"A": SH_A, "P": SH_P, "S": SH_S}[sh_key]
            wcache[key] = _unit_weights(kernels, grp[1], b, sh)
        return wcache[key]

    rt2 = np.sqrt(2.0, dtype=np.float32)
    in_maps = []
    metas = []
    for core in range(NCORES):
        rhsA_h = np.zeros((nA, SH_A["K"], RHS_F), BF16)
        wtsA_h = np.zeros((nA, SH_A["K"], len(SH_A["steps"]) * 32), BF16)
        rhsP_h = np.zeros((max(nP, 1), SH_P["K"], RHS_F), BF16)
        wtsP_h = np.zeros((max(nP, 1), SH_P["K"], len(SH_P["steps"]) * 32),
                          BF16)
        rhsS_h = np.zeros((max(nS, 1), SH_S["K"], RHS_F), BF16)
        wtsS_h = np.zeros((max(nS, 1), SH_S["K"], len(SH_S["steps"]) * 24),
                          BF16)
        par_h = np.zeros((128, 2 * NU), np.float32)
        meta = []

        def fill(slot, local, udesc, sh, rhs_h, wts_h):
            if udesc is None:
                meta.append(None)
                return
            _, (c, ks), x0, b = udesc
            rhs_h[local] = _unit_slab(padded[c], x0, sh)
            wts_h[local] = unit_w(udesc[0], udesc[1], x0, b)
            for q in range(NTILES):
                for ki, k in enumerate(ks):
                    sc = np.float32(1.0 / (rt2 * s[k]))
                    r0 = 32 * q + ki * b
                    par_h[r0:r0 + b, slot] = sc
                    par_h[r0:r0 + b, NU + slot] = -m[k] * sc
            meta.append(udesc)

        for j in range(nA):
            fill(j, j, unitsA[core * nA + j], SH_A, rhsA_h, wtsA_h)
        for j in range(nP):
            fill(nA + j, j, unitsP[core * nP + j], SH_P, rhsP_h, wtsP_h)
        for j in range(nS):
            fill(nA + nP + j, j, unitsS[core * nS + j], SH_S,
                 rhsS_h, wtsS_h)
        in_maps.append({"rhsA": rhsA_h, "wtsA": wtsA_h,
                        "rhsP": rhsP_h, "wtsP": wtsP_h,
                        "rhsS": rhsS_h, "wtsS": wtsS_h, "par": par_h})
        metas.append(meta)

    key = (nA, nP, nS)
    if key not in _NC_CACHE:
        _NC_CACHE[key] = _build_nc(nA, max(nP, 1), max(nS, 1))
    nc = _NC_CACHE[key]

    import os
    prof_dir = os.environ.get("KERNEL_PROFILE_DIR")
    if prof_dir:
        from trn_agent_boot.trn_boot import _ntff_profile_via_ctypes
        hook = _ntff_profile_via_ctypes("/opt/axon/libaxon_pjrt.so")
        with hook(prof_dir, [0]):
            res = run_bass_kernel_spmd(nc, in_maps,
                                       core_ids=list(range(NCORES)))
    else:
        res = run_bass_kernel_spmd(nc, in_maps, core_ids=list(range(NCORES)))
    global LAST_EXEC_NS
    LAST_EXEC_NS = res.exec_time_ns

    field = np.zeros((C, S, S, S), np.float32)      # [c, X, Y, Z]
    for core in range(NCORES):
        g0 = res.results[core]["g0"]                # [NU, 128, 2304]
        for j, mt in enumerate(metas[core]):
            if mt is None:
                continue
            _, (c, ks), x0, b = mt
            blk = g0[j].reshape(128, YQ, S)
            for q in range(NTILES):
                for ki, k in enumerate(ks):
                    r0 = 32 * q + ki * b
                    field[c1_idx[k], x0:x0 + b, q::NTILES] += \
                        2.0 * blk[r0:r0 + b] - 1.0

    out = input + field.transpose(1, 2, 3, 0)[None] / T[0]
    return np.clip(out, 0.0, MAXP).astype(np.float32)


# revision 12
# speedup vs baseline: 2.6614x; 1.0017x over previous
"""Trainium2 Bass kernel for nn_CppnPotentialCAStep — col-tiled v2.

Reference computation (per kernel k of NK=32):
  pot_k = depthwise_conv3d_wrap(x[:, :, :, c0[k]], kernels[k])    # 15^3 taps
  g_k   = exp(-(pot_k - m[k])^2 / (2 s[k]^2)) * 2 - 1
  field[c] = sum_{k: c1[k]==c} g_k
  out = clip(input + field / T, 0, 10)

Mapping: banded-Toeplitz over X (band in a b+14-row window) with ns
z-shifted copies of the channel stacked in the contraction dim, as in
v1 — but every matmul now runs in PE array mode (128, 32): 4 column
tiles execute concurrently, each streaming its own moving operand.
The 4 tiles process the 4 y-quarters (24 rows each) of the same
(channel-group, x-chunk) unit, sharing one SBUF slab and accumulating
into the 4 partition-quadrants of a shared PSUM bank, so one Square+
Exp activation pass and one output DMA cover all 4 tiles.

Unit shapes (c0 multiplicity histogram {1:6, 2:3, 3:3, 4:1, 7:1};
the 7-kernel channel is split 4+3):
  A: quads b=8 / triples b=10(+tail b=6): K=5x24=120, M=32, 45 steps
     (shifts 0,3,6,9,12; j=0..2), 64 units -> 8 per core
  P: pairs b=16: K=4x30=120, M=32, 60 steps (shifts 0,4,8,12; j=0..3),
     18 real + 6 dummy units -> 3 per core
  S: singles b=24: K=3x38=114, M=24, 75 steps (shifts 0,5,10; j=0..4),
     24 units -> 3 per core
Per core: 765 accumulation steps x 5 psum rounds x 4 concurrent tiles.
"""

import numpy as np
import ml_dtypes

BF16 = ml_dtypes.bfloat16

S = 96          # grid size
C = 16          # channels
KS = 15         # kernel taps per axis
PAD = 7
MAXP = 10.0

XP = 114        # padded x extent: orig -7 .. 106  (covers window 24 at x0=90)
YP = 110        # padded y extent: orig -7 .. 102
ZPH = 122       # padded z extent: orig -7 .. 114  (covers shift 12 + 110)
ZS = 110        # slab z extent per block
RHS_F = YP * ZS  # 12100 free elements per slab partition row
NCORES = 8
NTILES = 4      # column tiles (PE mode (128, 32))
YQ = 24         # y rows per quarter/tile
ROUNDS = [(0, 5), (5, 5), (10, 5), (15, 5), (20, 4)]  # (y0, ny) per quarter

# shape tables: (window_rows, n_blocks, shifts, t, M)
SH_A = dict(W=24, shifts=(0, 3, 6, 9, 12), t=3, M=32, K=120)
SH_P = dict(W=30, shifts=(0, 4, 8, 12), t=4, M=32, K=120)
SH_S = dict(W=38, shifts=(0, 5, 10), t=5, M=24, K=114)
for _sh in (SH_A, SH_P, SH_S):
    _sh["steps"] = [(dy, j) for dy in range(KS) for j in range(_sh["t"])]


def _build_groups(c0_idx):
    """Group kernels by source channel: quads+triples (A), pairs (P),
    singles (S). Multiplicity >=4 is peeled into quads."""
    by_ch = {}
    for k, c in enumerate(c0_idx):
        by_ch.setdefault(int(c), []).append(k)
    ga, gp, gs = [], [], []
    for c in sorted(by_ch):
        ks = by_ch[c]
        while len(ks) >= 4:
            ga.append((c, ks[:4]))
            ks = ks[4:]
        if len(ks) == 3:
            ga.append((c, ks))
        elif len(ks) == 2:
            gp.append((c, ks))
        elif len(ks) == 1:
            gs.append((c, ks))
    return ga, gp, gs


def _chunks_for(shape_key, g):
    """X-chunk widths for a group of g kernels under a shape."""
    if shape_key == "A":
        b = 8 if g == 4 else 10
        out = []
        x = 0
        while x < S:
            w = min(b, S - x)
            out.append((x, w))
            x += w
        return out                       # g=4: 12x8; g=3: 9x10 + 1x6
    if shape_key == "P":
        return [(16 * i, 16) for i in range(6)]
    return [(24 * i, 24) for i in range(4)]


def _unit_weights(kernels, ks, x_b, sh):
    """Stationary weights [K, steps*M] for one (group, chunk) unit."""
    W, shifts, t, M, K = sh["W"], sh["shifts"], sh["t"], sh["M"], sh["K"]
    steps = sh["steps"]
    b = x_b
    out = np.zeros((K, len(steps), M), np.float32)
    for i, (dy, j) in enumerate(steps):
        for blk, s in enumerate(shifts):
            dz = s + j
            if dz >= KS:
                continue
            for ki, k in enumerate(ks):
                v = kernels[k][:, dy, dz]           # [15] over dx
                for col in range(b):
                    out[blk * W + col:blk * W + col + KS, i, ki * b + col] = v
    return out.reshape(K, len(steps) * M).astype(BF16)


def _unit_slab(padded_c, x0, sh):
    """Moving slab [K, 110*110]: n_blocks z-shifted x-window copies."""
    W, shifts, K = sh["W"], sh["shifts"], sh["K"]
    out = np.empty((K, RHS_F), BF16)
    for blk, s in enumerate(shifts):
        out[blk * W:(blk + 1) * W] = \
            padded_c[x0:x0 + W, :, s:s + ZS].reshape(W, RHS_F)
    return out


def _build_nc(nA, nP, nS):
    import concourse.bass as bass  # noqa: F401
    import concourse.mybir as mybir
    from concourse import bacc
    from concourse.tile import TileContext

    nc = bacc.Bacc(None, target_bir_lowering=False)
    NU = nA + nP + nS
    rhsA = nc.dram_tensor("rhsA", [nA, SH_A["K"], RHS_F],
                          mybir.dt.bfloat16, kind="ExternalInput")
    wtsA = nc.dram_tensor("wtsA", [nA, SH_A["K"], len(SH_A["steps"]) * 32],
                          mybir.dt.bfloat16, kind="ExternalInput")
    rhsP = nc.dram_tensor("rhsP", [nP, SH_P["K"], RHS_F],
                          mybir.dt.bfloat16, kind="ExternalInput")
    wtsP = nc.dram_tensor("wtsP", [nP, SH_P["K"], len(SH_P["steps"]) * 32],
                          mybir.dt.bfloat16, kind="ExternalInput")
    rhsS = nc.dram_tensor("rhsS", [nS, SH_S["K"], RHS_F],
                          mybir.dt.bfloat16, kind="ExternalInput")
    wtsS = nc.dram_tensor("wtsS", [nS, SH_S["K"], len(SH_S["steps"]) * 24],
                          mybir.dt.bfloat16, kind="ExternalInput")
    par_in = nc.dram_tensor("par", [128, 2 * NU],
                            mybir.dt.float32, kind="ExternalInput")
    g0_out = nc.dram_tensor("g0", [NU, 128, YQ * S],
                            mybir.dt.float32, kind="ExternalOutput")
    AF = mybir.ActivationFunctionType

    with TileContext(nc) as tc:
        with tc.tile_pool(name="rhsp", bufs=2) as rhsp, \
             tc.tile_pool(name="wp", bufs=2) as wp, \
             tc.tile_pool(name="parp", bufs=1) as parp, \
             tc.tile_pool(name="psp", bufs=4, space="PSUM") as psp, \
             tc.tile_pool(name="gp", bufs=4) as gp:
            par_t = parp.tile([128, 2 * NU], mybir.dt.float32)
            nc.sync.dma_start(out=par_t, in_=par_in[:])

            def load(rhs_ext, wts_ext, sh):
                K, M = sh["K"], sh["M"]
                nst = len(sh["steps"])
                rhs_t = rhsp.tile([K, RHS_F], mybir.dt.bfloat16, tag="rhs")
                w_t = wp.tile([K, nst * M], mybir.dt.bfloat16, tag="wts")
                wq = (nst // 5) * M
                # chunked loads on both hwdge queues; with the y-mod-4
                # quadrant interleave, psum round r only needs y rows up
                # to 20r+33, so compute starts after the first two slab
                # chunks plus the first weight chunk land
                ys = (0, 18, 34, 54, 74, 94, YP)
                for ci in range(2):
                    a, b = ys[ci], ys[ci + 1]
                    eng = nc.scalar if ci % 2 else nc.sync
                    eng.dma_start(out=rhs_t[:, a * ZS:b * ZS],
                                  in_=rhs_ext[:, a * ZS:b * ZS])
                nc.sync.dma_start(out=w_t[:, :wq], in_=wts_ext[:, :wq])
                for q in range(1, 5):
                    eng = nc.scalar if q % 2 else nc.sync
                    eng.dma_start(out=w_t[:, q * wq:(q + 1) * wq],
                                  in_=wts_ext[:, q * wq:(q + 1) * wq])
                for ci in range(2, 6):
                    a, b = ys[ci], ys[ci + 1]
                    eng = nc.scalar if ci % 2 else nc.sync
                    eng.dma_start(out=rhs_t[:, a * ZS:b * ZS],
                                  in_=rhs_ext[:, a * ZS:b * ZS])
                return rhs_t, w_t

            def rounds(u, handles, sh, r_list):
                K, M = sh["K"], sh["M"]
                steps = sh["steps"]
                rhs_t, w_t = handles
                rhs3 = rhs_t.rearrange("p (y z) -> p y z", z=ZS)
                last = len(steps) - 1
                for y0, ny in r_list:
                    nt = ny * S
                    ps_t = psp.tile([128, nt], mybir.dt.float32, tag="ps")
                    for i, (dy, j) in enumerate(steps):
                        lhsT = w_t[:, i * M:(i + 1) * M]
                        for q in range(NTILES):
                            # quadrant q covers output y rows q, q+4, ...
                            yb = 4 * y0 + q + dy
                            nc.tensor.matmul(
                                ps_t[32 * q:32 * q + M, :],
                                lhsT=lhsT,
                                rhs=rhs3[:, yb:yb + 4 * ny - 3:4,
                                         j:j + S],
                                start=(i == 0),
                                stop=(i == last),
                                tile_position=(0, 32 * q),
                            )
                    sq_t = gp.tile([128, nt], mybir.dt.float32, tag="sq")
                    nc.scalar.activation(
                        sq_t, ps_t, AF.Square,
                        bias=par_t[:, NU + u:NU + u + 1],
                        scale=par_t[:, u:u + 1])
                    g0_t = gp.tile([128, nt], mybir.dt.float32, tag="g0")
                    nc.scalar.activation(g0_t, sq_t, AF.Exp, scale=-1.0)
                    nc.sync.dma_start(
                        out=g0_out[u, :, y0 * S:(y0 + ny) * S], in_=g0_t)

            seq = [(rhsA[s], wtsA[s], SH_A) for s in range(nA)] + \
                  [(rhsP[s], wtsP[s], SH_P) for s in range(nP)] + \
                  [(rhsS[s], wtsS[s], SH_S) for s in range(nS)]
            handles = load(*seq[0])
            for u in range(len(seq)):
                sh = seq[u][2]
                # first psum round, then prefetch the next unit's data
                # while the remaining rounds keep the PE busy
                rounds(u, handles, sh, ROUNDS[:1])
                nxt = load(*seq[u + 1]) if u + 1 < len(seq) else None
                rounds(u, handles, sh, ROUNDS[1:])
                handles = nxt
    nc.finalize()
    return nc


_NC_CACHE = {}
LAST_EXEC_NS = None


def kernel(input, kernels, m, s, T, c0_idx, c1_idx):
    from concourse.bass_utils import run_bass_kernel_spmd

    input = np.asarray(input, np.float32)
    kernels = np.asarray(kernels, np.float32)
    m = np.asarray(m, np.float32)
    s = np.asarray(s, np.float32)
    T = np.asarray(T, np.float32)
    c0_idx = np.asarray(c0_idx)
    c1_idx = np.asarray(c1_idx)

    x = input[0].transpose(3, 0, 1, 2)          # [C, X, Y, Z]
    ga, gp_, gs = _build_groups(c0_idx)

    # units: (shape_key, group, x0, b) in a fixed global order
    unitsA = [("A", g, x0, b) for g in ga for x0, b in _chunks_for("A", len(g[1]))]
    unitsP = [("P", g, x0, b) for g in gp_ for x0, b in _chunks_for("P", 2)]
    unitsS = [("S", g, x0, b) for g in gs for x0, b in _chunks_for("S", 1)]
    for lst in (unitsA, unitsP, unitsS):
        while len(lst) % NCORES:
            lst.append(None)                     # dummy unit (zeros)
    nA = len(unitsA) // NCORES
    nP = len(unitsP) // NCORES
    nS = len(unitsS) // NCORES
    NU = nA + nP + nS

    # wrap-padded channels [114, 110, 122]
    ix = (np.arange(XP) - PAD) % S
    iy = (np.arange(YP) - PAD) % S
    iz = (np.arange(ZPH) - PAD) % S
    used = {g[0] for g in ga + gp_ + gs}
    padded = {c: x[c][ix][:, iy][:, :, iz].astype(BF16) for c in used}

    wcache = {}

    def unit_w(sh_key, grp, x0, b):
        key = (sh_key, grp[0], tuple(grp[1]), b)
        if key not in wcache:
            sh = {"A": SH_A, "P": SH_P, "S": SH_S}[sh_key]
            wcache[key] = _unit_weights(kernels, grp[1], b, sh)
        return wcache[key]

    rt2 = np.sqrt(2.0, dtype=np.float32)
    in_maps = []
    metas = []
    for core in range(NCORES):
        rhsA_h = np.zeros((nA, SH_A["K"], RHS_F), BF16)
        wtsA_h = np.zeros((nA, SH_A["K"], len(SH_A["steps"]) * 32), BF16)
        rhsP_h = np.zeros((max(nP, 1), SH_P["K"], RHS_F), BF16)
        wtsP_h = np.zeros((max(nP, 1), SH_P["K"], len(SH_P["steps"]) * 32),
                          BF16)
        rhsS_h = np.zeros((max(nS, 1), SH_S["K"], RHS_F), BF16)
        wtsS_h = np.zeros((max(nS, 1), SH_S["K"], len(SH_S["steps"]) * 24),
                          BF16)
        par_h = np.zeros((128, 2 * NU), np.float32)
        meta = []

        def fill(slot, local, udesc, sh, rhs_h, wts_h):
            if udesc is None:
                meta.append(None)
                return
            _, (c, ks), x0, b = udesc
            rhs_h[local] = _unit_slab(padded[c], x0, sh)
            wts_h[local] = unit_w(udesc[0], udesc[1], x0, b)
            for q in range(NTILES):
                for ki, k in enumerate(ks):
                    sc = np.float32(1.0 / (rt2 * s[k]))
                    r0 = 32 * q + ki * b
                    par_h[r0:r0 + b, slot] = sc
                    par_h[r0:r0 + b, NU + slot] = -m[k] * sc
            meta.append(udesc)

        for j in range(nA):
            fill(j, j, unitsA[core * nA + j], SH_A, rhsA_h, wtsA_h)
        for j in range(nP):
            fill(nA + j, j, unitsP[core * nP + j], SH_P, rhsP_h, wtsP_h)
        for j in range(nS):
            fill(nA + nP + j, j, unitsS[core * nS + j], SH_S,
                 rhsS_h, wtsS_h)
        in_maps.append({"rhsA": rhsA_h, "wtsA": wtsA_h,
                        "rhsP": rhsP_h, "wtsP": wtsP_h,
                        "rhsS": rhsS_h, "wtsS": wtsS_h, "par": par_h})
        metas.append(meta)

    key = (nA, nP, nS)
    if key not in _NC_CACHE:
        _NC_CACHE[key] = _build_nc(nA, max(nP, 1), max(nS, 1))
    nc = _NC_CACHE[key]

    import os
    prof_dir = os.environ.get("KERNEL_PROFILE_DIR")
    if prof_dir:
        from trn_agent_boot.trn_boot import _ntff_profile_via_ctypes
        hook = _ntff_profile_via_ctypes("/opt/axon/libaxon_pjrt.so")
        with hook(prof_dir, [0]):
            res = run_bass_kernel_spmd(nc, in_maps,
                                       core_ids=list(range(NCORES)))
    else:
        res = run_bass_kernel_spmd(nc, in_maps, core_ids=list(range(NCORES)))
    global LAST_EXEC_NS
    LAST_EXEC_NS = res.exec_time_ns

    field = np.zeros((C, S, S, S), np.float32)      # [c, X, Y, Z]
    for core in range(NCORES):
        g0 = res.results[core]["g0"]                # [NU, 128, 2304]
        for j, mt in enumerate(metas[core]):
            if mt is None:
                continue
            _, (c, ks), x0, b = mt
            blk = g0[j].reshape(128, YQ, S)
            for q in range(NTILES):
                for ki, k in enumerate(ks):
                    r0 = 32 * q + ki * b
                    field[c1_idx[k], x0:x0 + b, q::NTILES] += \
                        2.0 * blk[r0:r0 + b] - 1.0

    out = input + field.transpose(1, 2, 3, 0)[None] / T[0]
    return np.clip(out, 0.0, MAXP).astype(np.float32)
